# revision 1
# baseline (speedup 1.0000x reference)
"""Trainium2 Bass kernel for nn_PredicateTensorModel.

Math (reference):
  subj/verb/obj[c,d] = weighted embedding bags (N=8 ids per batch row)
  A[c,p,q]  = sum_i w[i,p,q] verb[c,i]
  US[c,p,q] = sum_j u[j,p,q] subj[c,j]
  out[c,q]  = sum_p US[c,p,q] * A[c,p,q] * obj[c,p]

Sharding: tensor-parallel over trailing q axis (32 q's per core, 8 cores).
w/u are pre-transposed on host to [i, q, p] layout and sharded contiguous;
all TensorEngine-facing data is bf16 (f32 PSUM accumulation).

Embedding bags are computed on-device as matmuls: gathered rows for a
16-batch chunk sit on 128 partitions [(c,n), d]; a host-built sparse
[128, 16] block "S" holds the bag weights so  V.T @ S  yields the
transposed embedding [d, c16] directly (the layout the big matmuls need
as stationary operand).

Final contraction: D = A*US (DVE), then scalar_tensor_tensor with
accum_out fuses (D * obj) and the sum over p into one DVE op per q.
"""

import os
import sys

sys.path.insert(0, "/opt/trn_rl_repo")

import numpy as np
import ml_dtypes

N_CORES = 8
VOCAB, D, B, N = 50000, 256, 512, 8
QS = D // N_CORES  # 32 q columns per core
NCHUNK = B // 16  # 32 gather chunks of 16 batch rows

bf16 = ml_dtypes.bfloat16

_PROG_CACHE = {}


def _build_program():
    import concourse.bass as bass
    import concourse.tile as tile
    import concourse.mybir as mybir
    from concourse import bacc
    from contextlib import ExitStack

    dt = mybir.dt
    nc = bacc.Bacc()

    emb_p = nc.declare_dram_parameter("emb_b", [VOCAB, D], dt.bfloat16, isOutput=False)
    w_p = nc.declare_dram_parameter("w_k", [D, QS, D], dt.bfloat16, isOutput=False)
    u_p = nc.declare_dram_parameter("u_k", [D, QS, D], dt.bfloat16, isOutput=False)
    ids_p = {}
    S_p = {}
    for t in "svo":
        ids_p[t] = nc.declare_dram_parameter(f"ids_{t}", [128, NCHUNK], dt.int32, isOutput=False)
        S_p[t] = nc.declare_dram_parameter(f"S_{t}", [128, B], dt.bfloat16, isOutput=False)
    ident_p = nc.declare_dram_parameter("ident", [128, 128], dt.bfloat16, isOutput=False)
    out_p = nc.declare_dram_parameter("out", [B, QS], dt.float32, isOutput=True)

    with ExitStack() as ctx:
        tc = ctx.enter_context(tile.TileContext(nc))
        const_pool = ctx.enter_context(tc.tile_pool(name="const", bufs=1))
        gather_pool = ctx.enter_context(tc.tile_pool(name="gather", bufs=12))
        embT_pool = ctx.enter_context(tc.tile_pool(name="embT", bufs=1))
        w_pool = ctx.enter_context(tc.tile_pool(name="wtiles", bufs=8))
        stage_pool = ctx.enter_context(tc.tile_pool(name="stage", bufs=4))
        out_pool = ctx.enter_context(tc.tile_pool(name="outp", bufs=1))
        psum_pool = ctx.enter_context(tc.tile_pool(name="ps", bufs=8, space="PSUM"))

        # ---- constants ----
        idt = {}
        St = {}
        for t in "svo":
            idt[t] = const_pool.tile([128, NCHUNK], dt.int32, name=f"ids{t}", tag=f"ids{t}")
            nc.sync.dma_start(out=idt[t][:], in_=ids_p[t][:])
            St[t] = const_pool.tile([128, B], dt.bfloat16, name=f"S{t}", tag=f"S{t}")
            nc.sync.dma_start(out=St[t][:], in_=S_p[t][:])
        ident = const_pool.tile([128, 128], dt.bfloat16, name="ident", tag="ident")
        nc.sync.dma_start(out=ident[:], in_=ident_p[:])
        dummy = const_pool.tile([1, 1], dt.int32, name="dummy", tag="dummy")
        # Warm the gpsimd engine clock on all three ids loads up front, so no
        # later indirect DMA needs a second sync wait for them.
        for t in "svo":
            nc.gpsimd.tensor_copy(dummy[:], idt[t][:1, :1])

        # ---- phase E: embedding bags -> transposed [d, c] bf16 tiles ----
        # psT[t][dh] accumulates [128 d, 512 c] f32 column blocks
        psT = {}
        embT = {}
        for t in "svo":
            psT[t] = [psum_pool.tile([128, B], dt.float32, name="psT", tag="ps") for _ in range(2)]
            for ck in range(NCHUNK):
                V = gather_pool.tile([128, D], dt.bfloat16, name="V", tag="V")
                nc.gpsimd.indirect_dma_start(
                    out=V[:],
                    out_offset=None,
                    in_=emb_p[:],
                    in_offset=bass.IndirectOffsetOnAxis(
                        ap=idt[t][:, ck : ck + 1], axis=0
                    ),
                )
                for dh in range(2):
                    nc.tensor.matmul(
                        out=psT[t][dh][:, ck * 16 : (ck + 1) * 16],
                        lhsT=V[:, dh * 128 : (dh + 1) * 128],
                        rhs=St[t][:, ck * 16 : (ck + 1) * 16],
                        start=True,
                        stop=True,
                    )
            embT[t] = []
            for dh in range(2):
                e = embT_pool.tile([128, B], dt.bfloat16, name=f"eT{t}{dh}", tag=f"eT{t}{dh}")
                nc.scalar.copy(out=e[:], in_=psT[t][dh][:])
                embT[t].append(e)

        # obj needs the untransposed [c, p] layout: transpose objT back via PE
        obj_s = [embT_pool.tile([128, D], dt.bfloat16, name=f"objs{ch}", tag=f"obj{ch}") for ch in range(4)]
        for dh in range(2):
            for ch in range(4):
                ptr = psum_pool.tile([128, 128], dt.bfloat16, name="ptr", tag="ps")
                nc.tensor.transpose(
                    out=ptr[:],
                    in_=embT["o"][dh][:, ch * 128 : (ch + 1) * 128],
                    identity=ident[:],
                )
                nc.scalar.copy(out=obj_s[ch][:, dh * 128 : (dh + 1) * 128], in_=ptr[:])

        # ---- phase M: big matmuls + fused elementwise/reduce ----
        outs = [out_pool.tile([128, QS], dt.float32, name=f"outs{ck}", tag=f"out{ck}") for ck in range(4)]
        for qj in range(QS // 2):  # 16 q-pairs
            wt = []
            ut = []
            for ic in range(2):
                wtile = w_pool.tile([128, 512], dt.bfloat16, name="wtile", tag="wt")
                nc.sync.dma_start(
                    out=wtile[:],
                    in_=w_p[ic * 128 : (ic + 1) * 128, qj * 2 : qj * 2 + 2, :],
                )
                wt.append(wtile)
                utile = w_pool.tile([128, 512], dt.bfloat16, name="utile", tag="ut")
                nc.sync.dma_start(
                    out=utile[:],
                    in_=u_p[ic * 128 : (ic + 1) * 128, qj * 2 : qj * 2 + 2, :],
                )
                ut.append(utile)
            for ck in range(4):
                psA = psum_pool.tile([128, 512], dt.float32, name="psM", tag="ps")
                psU = psum_pool.tile([128, 512], dt.float32, name="psM", tag="ps")
                for ic in range(2):
                    nc.tensor.matmul(
                        out=psA[:],
                        lhsT=embT["v"][ic][:, ck * 128 : (ck + 1) * 128],
                        rhs=wt[ic][:],
                        start=(ic == 0),
                        stop=(ic == 1),
                    )
                for ic in range(2):
                    nc.tensor.matmul(
                        out=psU[:],
                        lhsT=embT["s"][ic][:, ck * 128 : (ck + 1) * 128],
                        rhs=ut[ic][:],
                        start=(ic == 0),
                        stop=(ic == 1),
                    )
                USs = stage_pool.tile([128, 512], dt.bfloat16, name="USs", tag="USs")
                nc.scalar.copy(out=USs[:], in_=psU[:])
                Dt = stage_pool.tile([128, 512], dt.bfloat16, name="Dt", tag="D")
                nc.vector.tensor_mul(Dt[:], psA[:], USs[:])
                junk = stage_pool.tile([128, D], dt.bfloat16, name="junk", tag="junk")
                for qq in range(2):
                    q_col = qj * 2 + qq
                    nc.vector.scalar_tensor_tensor(
                        out=junk[:],
                        in0=Dt[:, qq * D : (qq + 1) * D],
                        scalar=1.0,
                        in1=obj_s[ck][:],
                        op0=mybir.AluOpType.mult,
                        op1=mybir.AluOpType.mult,
                        accum_out=outs[ck][:, q_col : q_col + 1],
                    )
        for ck in range(4):
            nc.sync.dma_start(
                out=out_p[ck * 128 : (ck + 1) * 128, :], in_=outs[ck][:]
            )

    nc.finalize()
    return nc


def _get_program():
    if "nc" not in _PROG_CACHE:
        _PROG_CACHE["nc"] = _build_program()
    return _PROG_CACHE["nc"]


def _host_prep(inputs):
    """Shard + lay out inputs for the 8 cores. Returns list of in_maps."""
    ids = {}
    wts = {}
    for t, idk, wk in (
        ("s", "subj_id", "subj_w"),
        ("v", "verb_id", "verb_w"),
        ("o", "obj_id", "obj_w"),
    ):
        ids[t] = np.asarray(inputs[idk]).astype(np.int32)
        wts[t] = np.asarray(inputs[wk]).astype(np.float32)

    emb = np.asarray(inputs["emb"], dtype=np.float32)
    w = np.asarray(inputs["w"], dtype=np.float32)
    u = np.asarray(inputs["u"], dtype=np.float32)

    emb_b = emb.astype(bf16)
    # [i, p, q] -> [i, q, p], contiguous, then shard q
    wT = np.ascontiguousarray(w.transpose(0, 2, 1)).astype(bf16)
    uT = np.ascontiguousarray(u.transpose(0, 2, 1)).astype(bf16)

    ids_r = {}
    S_m = {}
    for t in "svo":
        # partition p = (c % 16)*8 + n ; column = chunk ck = c // 16
        ids_r[t] = np.ascontiguousarray(
            ids[t].reshape(NCHUNK, 16, 8).transpose(1, 2, 0).reshape(128, NCHUNK)
        )
        Sm = np.zeros((16, 8, NCHUNK, 16), np.float32)
        wr = wts[t].reshape(NCHUNK, 16, 8).transpose(1, 2, 0)  # [16 j, 8 n, 32 ck]
        j = np.arange(16)
        Sm[j[:, None, None], np.arange(8)[None, :, None], np.arange(NCHUNK)[None, None, :], j[:, None, None]] = wr
        S_m[t] = np.ascontiguousarray(Sm.reshape(128, B)).astype(bf16)

    ident = np.eye(128, dtype=bf16)

    in_maps = []
    for k in range(N_CORES):
        m = {
            "emb_b": emb_b,
            "w_k": np.ascontiguousarray(wT[:, k * QS : (k + 1) * QS, :]),
            "u_k": np.ascontiguousarray(uT[:, k * QS : (k + 1) * QS, :]),
            "ident": ident,
        }
        for t in "svo":
            m[f"ids_{t}"] = ids_r[t]
            m[f"S_{t}"] = S_m[t]
        in_maps.append(m)
    return in_maps


def kernel(**inputs) -> np.ndarray:
    from concourse.bass_utils import run_bass_kernel_spmd

    nc = _get_program()
    in_maps = _host_prep(inputs)
    trace = bool(int(os.environ.get("KTRACE", "0")))
    res = run_bass_kernel_spmd(
        nc, in_maps, core_ids=list(range(N_CORES)), trace=trace
    )
    if trace:
        _PROG_CACHE["last_result"] = res
    out = np.concatenate(
        [res.results[k]["out"].astype(np.float32) for k in range(N_CORES)], axis=1
    )
    return out



# revision 22
# speedup vs baseline: 1.5332x; 1.5332x over previous
"""Trainium2 Bass kernel for nn_PredicateTensorModel.

Math (reference):
  subj/verb/obj[c,d] = weighted embedding bags (N=8 ids per batch row)
  A[c,p,q]  = sum_i w[i,p,q] verb[c,i]
  US[c,p,q] = sum_j u[j,p,q] subj[c,j]
  out[c,q]  = sum_p US[c,p,q] * A[c,p,q] * obj[c,p]

Sharding: tensor-parallel over trailing q axis (32 q's per core, 8 cores).
w/u are pre-transposed on host to [i, q, p] layout and sharded contiguous;
all TensorEngine-facing data is bf16 (f32 PSUM accumulation).

Structure (v2):
  - Embedding rows are fetched with 12 large indirect DMAs (1024 rows
    each: one c-quarter per tensor), not per-16-batch chunks: SWDGE
    descriptor generation on the Pool engine is ~1us fixed per indirect
    DMA, so few big gathers beat many small ones.
  - w/u slices live fully resident in SBUF ([128, 8192] bf16 tiles),
    loaded with 16 [128, 2048] DMAs to keep HWDGE overhead low.
  - Per (c-block, q-pair) tile: PE computes psA/psU; Act copies psU to
    bf16 SBUF; G = USs*obj runs on gpsimd (Pool) for most tiles and DVE
    for the rest (load balance); DVE folds psA in with an accumulating
    scalar_tensor_tensor that also reduces over p.
"""

import os
import sys

sys.path.insert(0, "/opt/trn_rl_repo")

import numpy as np
import ml_dtypes

N_CORES = 8
VOCAB, D, B, N = 50000, 256, 512, 8
QS = D // N_CORES  # 32 q columns per core
NCHUNK = B // 16  # 32 gather chunks of 16 batch rows
NQUART = 4  # c-quarters (128 batch rows each)
CHQ = NCHUNK // NQUART  # 8 chunks per quarter
CHH = NCHUNK // 2  # 16 chunks per gather half

bf16 = ml_dtypes.bfloat16

_PROG_CACHE = {}

# Fraction control: tiles with (index % 8) < POOL_MOD run the G multiply
# on gpsimd; the rest on DVE.
POOL_MOD = 5


def _build_program():
    import concourse.bass as bass
    import concourse.tile as tile
    import concourse.mybir as mybir
    from concourse import bacc
    from contextlib import ExitStack

    dt = mybir.dt
    nc = bacc.Bacc()

    emb_p = nc.declare_dram_parameter("emb_b", [VOCAB, D], dt.bfloat16, isOutput=False)
    w_p = nc.declare_dram_parameter("w_k", [D, QS, D], dt.bfloat16, isOutput=False)
    u_p = nc.declare_dram_parameter("u_k", [D, QS, D], dt.bfloat16, isOutput=False)
    ids_p = {}
    S_p = {}
    for t in "svo":
        ids_p[t] = nc.declare_dram_parameter(f"ids_{t}", [128, NCHUNK], dt.int32, isOutput=False)
        S_p[t] = nc.declare_dram_parameter(f"S_{t}", [128, B], dt.bfloat16, isOutput=False)
    ident_p = nc.declare_dram_parameter("ident", [128, 128], dt.bfloat16, isOutput=False)
    out_p = nc.declare_dram_parameter("out", [B, QS], dt.float32, isOutput=True)

    with ExitStack() as ctx:
        tc = ctx.enter_context(tile.TileContext(nc))
        const_pool = ctx.enter_context(tc.tile_pool(name="const", bufs=1))
        gather_pool = ctx.enter_context(tc.tile_pool(name="gather", bufs=30))
        embT_pool = ctx.enter_context(tc.tile_pool(name="embT", bufs=1))
        wu_pool = ctx.enter_context(tc.tile_pool(name="wu", bufs=1))
        stage_pool = ctx.enter_context(tc.tile_pool(name="stage", bufs=6))
        out_pool = ctx.enter_context(tc.tile_pool(name="outp", bufs=1))
        psum_pool = ctx.enter_context(tc.tile_pool(name="ps", bufs=7, space="PSUM"))
        psum_small = ctx.enter_context(tc.tile_pool(name="pssm", bufs=1, space="PSUM"))

        # ---- constants ----
        idt = {}
        St = {}
        for t in "svo":
            idt[t] = const_pool.tile([128, NCHUNK], dt.int32, name=f"ids{t}", tag=f"ids{t}")
            nc.sync.dma_start(out=idt[t][:], in_=ids_p[t][:])
            St[t] = const_pool.tile([128, B], dt.bfloat16, name=f"S{t}", tag=f"S{t}")
            nc.sync.dma_start(out=St[t][:], in_=S_p[t][:])
        ident = const_pool.tile([128, 128], dt.bfloat16, name="ident", tag="ident")
        nc.sync.dma_start(out=ident[:], in_=ident_p[:])
        dummy = const_pool.tile([1, 1], dt.int32, name="dummy", tag="dummy")
        # Warm the gpsimd engine clock on all three ids loads up front, so no
        # later indirect DMA needs a second sync wait for them.
        for t in "svo":
            nc.gpsimd.tensor_copy(dummy[:], idt[t][:1, :1])

        # ---- resident w/u tiles ----
        # w_res[ic] holds w[ic*128:(ic+1)*128, :, :] as [128, 32q x 256p].
        w_res = []
        u_res = []
        for ic in range(2):
            w_res.append(wu_pool.tile([128, QS * D], dt.bfloat16, name=f"w{ic}", tag=f"w{ic}"))
            u_res.append(wu_pool.tile([128, QS * D], dt.bfloat16, name=f"u{ic}", tag=f"u{ic}"))

        def load_wu_quad(o4):
            # 4 q columns = 1024 elements for each of w0/w1/u0/u1
            for ic in range(2):
                nc.sync.dma_start(
                    out=w_res[ic][:, o4 * 1024 : (o4 + 1) * 1024],
                    in_=w_p[ic * 128 : (ic + 1) * 128, o4 * 4 : (o4 + 1) * 4, :],
                )
                nc.sync.dma_start(
                    out=u_res[ic][:, o4 * 1024 : (o4 + 1) * 1024],
                    in_=u_p[ic * 128 : (ic + 1) * 128, o4 * 4 : (o4 + 1) * 4, :],
                )

        # ---- phase E state ----
        embT = {
            t: [
                embT_pool.tile([128, B], dt.bfloat16, name=f"eT{t}{dh}", tag=f"eT{t}{dh}")
                for dh in range(2)
            ]
            for t in "sv"
        }
        embT_o = [
            embT_pool.tile([128, B], dt.bfloat16, name=f"eTo{dh}", tag=f"eTo{dh}")
            for dh in range(2)
        ]
        # obj stored doubled: [c, p] block twice along free axis, so the
        # G multiply is a single [128, 512] tensor_tensor per tile.
        obj_s = [
            embT_pool.tile([128, 2 * D], dt.bfloat16, name=f"objs{k}", tag=f"obj{k}")
            for k in range(NQUART)
        ]

        def gather_chunk(t, ck):
            # One offset per partition is all the HW indirect DMA supports:
            # each instruction gathers exactly 128 rows (one 16-batch chunk).
            V = gather_pool.tile([128, D], dt.bfloat16, name=f"V{t}{ck}", tag="V")
            nc.gpsimd.indirect_dma_start(
                out=V[:],
                out_offset=None,
                in_=emb_p[:],
                in_offset=bass.IndirectOffsetOnAxis(
                    ap=idt[t][:, ck : ck + 1], axis=0
                ),
            )
            return V

        def bags_quarter(k, Vs):
            # psE_a: four 128-col slots: (s,0) (s,1) (v,0) (v,1)
            psE_a = psum_pool.tile([128, 512], dt.float32, name="psEa", tag="ps")
            psE_b = psum_pool.tile([128, 512], dt.float32, name="psEb", tag="ps")
            for si, (t, dh) in enumerate(
                (("s", 0), ("s", 1), ("v", 0), ("v", 1))
            ):
                for c8 in range(CHQ):
                    nc.tensor.matmul(
                        out=psE_a[:, si * 128 + c8 * 16 : si * 128 + (c8 + 1) * 16],
                        lhsT=Vs[t][c8][:, dh * 128 : (dh + 1) * 128],
                        rhs=St[t][:, (k * CHQ + c8) * 16 : (k * CHQ + c8 + 1) * 16],
                        start=True,
                        stop=True,
                    )
            for dh in range(2):
                for c8 in range(CHQ):
                    nc.tensor.matmul(
                        out=psE_b[:, dh * 128 + c8 * 16 : dh * 128 + (c8 + 1) * 16],
                        lhsT=Vs["o"][c8][:, dh * 128 : (dh + 1) * 128],
                        rhs=St["o"][:, (k * CHQ + c8) * 16 : (k * CHQ + c8 + 1) * 16],
                        start=True,
                        stop=True,
                    )
            cb = k * 128
            for dh in range(2):
                nc.scalar.copy(out=embT["s"][dh][:, cb : cb + 128], in_=psE_a[:, dh * 128 : (dh + 1) * 128])
                nc.scalar.copy(out=embT["v"][dh][:, cb : cb + 128], in_=psE_a[:, 256 + dh * 128 : 256 + (dh + 1) * 128])
                nc.scalar.copy(out=embT_o[dh][:, cb : cb + 128], in_=psE_b[:, dh * 128 : (dh + 1) * 128])
            # transpose obj back to [c, p] for this c-quarter; store twice
            for dh in range(2):
                ptr = psum_small.tile([128, 128], dt.bfloat16, name="ptr", tag="pssm")
                nc.tensor.transpose(
                    out=ptr[:],
                    in_=embT_o[dh][:, cb : cb + 128],
                    identity=ident[:],
                )
                nc.scalar.copy(out=obj_s[k][:, dh * 128 : (dh + 1) * 128], in_=ptr[:])
                nc.scalar.copy(out=obj_s[k][:, D + dh * 128 : D + (dh + 1) * 128], in_=ptr[:])

        outs = [
            out_pool.tile([128, QS], dt.float32, name=f"outs{k}", tag=f"out{k}")
            for k in range(NQUART)
        ]

        tile_idx = 0

        def phase_m_tile(k, qj):
            nonlocal tile_idx
            psA = psum_pool.tile([128, 512], dt.float32, name="psA", tag="ps")
            psU = psum_pool.tile([128, 512], dt.float32, name="psU", tag="ps")
            cb = k * 128
            for ic in range(2):
                nc.tensor.matmul(
                    out=psA[:],
                    lhsT=embT["v"][ic][:, cb : cb + 128],
                    rhs=w_res[ic][:, qj * 512 : (qj + 1) * 512],
                    start=(ic == 0),
                    stop=(ic == 1),
                )
            for ic in range(2):
                nc.tensor.matmul(
                    out=psU[:],
                    lhsT=embT["s"][ic][:, cb : cb + 128],
                    rhs=u_res[ic][:, qj * 512 : (qj + 1) * 512],
                    start=(ic == 0),
                    stop=(ic == 1),
                )
            USs = stage_pool.tile([128, 512], dt.bfloat16, name="USs", tag="USs")
            nc.scalar.copy(out=USs[:], in_=psU[:])
            G = stage_pool.tile([128, 512], dt.bfloat16, name="G", tag="G")
            # Pool is saturated with gather SWDGE generation until the last
            # quarter; only offload G there once the gathers have drained.
            use_pool = tile_idx >= 50 and (tile_idx % 2 == 0)
            eng = nc.gpsimd if use_pool else nc.vector
            eng.tensor_mul(G[:], USs[:], obj_s[k][:])
            junk = stage_pool.tile([128, D], dt.bfloat16, name="junk", tag="junk")
            for qq in range(2):
                q_col = qj * 2 + qq
                nc.vector.scalar_tensor_tensor(
                    out=junk[:],
                    in0=psA[:, qq * D : (qq + 1) * D],
                    scalar=1.0,
                    in1=G[:, qq * D : (qq + 1) * D],
                    op0=mybir.AluOpType.mult,
                    op1=mybir.AluOpType.mult,
                    accum_out=outs[k][:, q_col : q_col + 1],
                )
            tile_idx += 1

        # ---- emission: interleave DMA + compute in readiness order ----
        # Each engine executes its instruction stream in program order, so
        # the emission order must match the intended execution order, and
        # tile_wait_until staggers DMA issue so the (single, serial) DMA
        # device processes transfers roughly in the order compute consumes
        # them. Quarter 0/1 use per-quarter gathers (earlier bags0); the
        # second c-half uses one bigger gather per tensor (cheaper on the
        # Pool engine, off the critical path by then).
        # Gathers trickle out of the Pool engine at ~1us each (SWDGE fixed
        # cost); they are the critical path. wu loads are cheap on the DMA
        # device and can all go early. Quarter k's gathers are ordered
        # s, v (matmul inputs) then o (needed a bit later for the STT).
        Vq = {}
        for k in range(NQUART):
            Vq[k] = {
                t: [gather_chunk(t, k * CHQ + c8) for c8 in range(CHQ)]
                for t in "svo"
            }
            load_wu_quad(2 * k)
            load_wu_quad(2 * k + 1)

        # quarter-major: each c-quarter's 16 tiles run while the next
        # quarter's gathers stream on the Pool engine
        order = [(k, o) for k in range(NQUART) for o in range(4)]
        bags_done = set()
        done_q = [0] * NQUART
        for (k, o) in order:
            if k not in bags_done:
                bags_quarter(k, Vq[k])
                bags_done.add(k)
            for qj in range(o * 4, o * 4 + 4):
                phase_m_tile(k, qj)
            done_q[k] += 1
            if done_q[k] == 4:
                nc.sync.dma_start(
                    out=out_p[k * 128 : (k + 1) * 128, :], in_=outs[k][:]
                )

    nc.finalize()
    return nc


def _get_program():
    if "nc" not in _PROG_CACHE:
        _PROG_CACHE["nc"] = _build_program()
    return _PROG_CACHE["nc"]


def _host_prep(inputs):
    """Shard + lay out inputs for the 8 cores. Returns list of in_maps."""
    ids = {}
    wts = {}
    for t, idk, wk in (
        ("s", "subj_id", "subj_w"),
        ("v", "verb_id", "verb_w"),
        ("o", "obj_id", "obj_w"),
    ):
        ids[t] = np.asarray(inputs[idk]).astype(np.int32)
        wts[t] = np.asarray(inputs[wk]).astype(np.float32)

    emb = np.asarray(inputs["emb"], dtype=np.float32)
    w = np.asarray(inputs["w"], dtype=np.float32)
    u = np.asarray(inputs["u"], dtype=np.float32)

    emb_b = emb.astype(bf16)
    # [i, p, q] -> [i, q, p], contiguous, then shard q
    wT = np.ascontiguousarray(w.transpose(0, 2, 1)).astype(bf16)
    uT = np.ascontiguousarray(u.transpose(0, 2, 1)).astype(bf16)

    ids_r = {}
    S_m = {}
    for t in "svo":
        # partition p = (c % 16)*8 + n ; column = chunk ck = c // 16
        ids_r[t] = np.ascontiguousarray(
            ids[t].reshape(NCHUNK, 16, 8).transpose(1, 2, 0).reshape(128, NCHUNK)
        )
        Sm = np.zeros((16, 8, NCHUNK, 16), np.float32)
        wr = wts[t].reshape(NCHUNK, 16, 8).transpose(1, 2, 0)  # [16 j, 8 n, 32 ck]
        j = np.arange(16)
        Sm[j[:, None, None], np.arange(8)[None, :, None], np.arange(NCHUNK)[None, None, :], j[:, None, None]] = wr
        S_m[t] = np.ascontiguousarray(Sm.reshape(128, B)).astype(bf16)

    ident = np.eye(128, dtype=bf16)

    in_maps = []
    for k in range(N_CORES):
        m = {
            "emb_b": emb_b,
            "w_k": np.ascontiguousarray(wT[:, k * QS : (k + 1) * QS, :]),
            "u_k": np.ascontiguousarray(uT[:, k * QS : (k + 1) * QS, :]),
            "ident": ident,
        }
        for t in "svo":
            m[f"ids_{t}"] = ids_r[t]
            m[f"S_{t}"] = S_m[t]
        in_maps.append(m)
    return in_maps


def kernel(**inputs) -> np.ndarray:
    from concourse.bass_utils import run_bass_kernel_spmd

    nc = _get_program()
    in_maps = _host_prep(inputs)
    trace = bool(int(os.environ.get("KTRACE", "0")))
    res = run_bass_kernel_spmd(
        nc, in_maps, core_ids=list(range(N_CORES)), trace=trace
    )
    if trace:
        _PROG_CACHE["last_result"] = res
    out = np.concatenate(
        [res.results[k]["out"].astype(np.float32) for k in range(N_CORES)], axis=1
    )
    return out
